# revision 17
# baseline (speedup 1.0000x reference)
"""Trainium2 Bass kernel for nn_Coconut (6 latent passes + final pass GPT-2).

Zero-collective architecture on 8 cores:
  - Cores b and b+4 redundantly run the transformer for batch b with a
    KV-cache formulation: 64-token prefill, 5 single-token decodes, and a
    443-token final chunk (causal attention makes all other positions
    identical across passes, so the reference's 7 full passes collapse).
  - lm_head / logits / softmax stats are split by vocab half across the pair
    (core b: V[0:16000], core b+4: V[16000:32000]).
  - Activations are feature-major [128, 8, T]; all matmuls bf16 x bf16 with
    f32 accumulation; LN / softmax / residual math in f32.
  - Loss is combined on host from per-core (rowmax, sumexp) stats plus a
    label gather on the returned logits.
"""
import math
import numpy as np
import ml_dtypes

import concourse.bass as bass
import concourse.tile as tile
import concourse.mybir as mybir
from concourse import bacc
from concourse.bass_utils import run_bass_kernel_spmd
from concourse.masks import make_identity

F32 = mybir.dt.float32
BF16 = mybir.dt.bfloat16
AF = mybir.ActivationFunctionType

B, S, D, H, L, V = 4, 512, 1024, 16, 4, 32000
DH = D // H                  # 64
P = 128
ND = D // P                  # 8 d-tiles
NH1 = (4 * D) // P           # 32 mlp h-tiles
VH = V // 2                  # vocab half per core
VCHUNK = 500
NVC = VH // VCHUNK           # 32
NTT = S // P                 # 4 token-tiles for lm_head
LATENT_START, N_LATENT = 64, 6
TPRE = LATENT_START                         # 64
T0FIN = LATENT_START + N_LATENT - 1         # 69
TFIN = S - T0FIN                            # 443
GC1 = math.sqrt(0.044715)
GC2 = 2.0 * math.sqrt(2.0 / math.pi)
LN_EPS = 1e-5

# (t0, T) per pass; kv_end = t0 + T; scatter into col t0+T except final pass
PASSES = [(0, TPRE)] + [(63 + d, 1) for d in range(1, 6)] + [(T0FIN, TFIN)]


def build_device():
    nc = bacc.Bacc("TRN2", target_bir_lowering=False, debug=False, num_devices=8)

    # ---- DRAM parameters ----
    x0T = nc.declare_dram_parameter("x0T", [P, ND, S], F32, isOutput=False)
    posT = nc.declare_dram_parameter("posT", [P, ND, S], F32, isOutput=False)
    lnp = nc.declare_dram_parameter("lnp", [4 * L + 2, P, ND], F32, isOutput=False)
    aqkv = nc.declare_dram_parameter("aqkv", [L * ND, P, 3 * D], BF16, isOutput=False)
    ao = nc.declare_dram_parameter("ao", [L * ND, P, D], BF16, isOutput=False)
    a1 = nc.declare_dram_parameter("a1", [L * ND, P, 4 * D], BF16, isOutput=False)
    a2 = nc.declare_dram_parameter("a2", [L * NH1, P, D], BF16, isOutput=False)
    lmw = nc.declare_dram_parameter("lmw", [ND, P, VH], BF16, isOutput=False)
    mpre = nc.declare_dram_parameter("mpre", [TPRE, TPRE], BF16, isOutput=False)
    ind2h = nc.declare_dram_parameter("ind2h", [2, P], F32, isOutput=False)
    mfin = nc.declare_dram_parameter("mfin", [4, P, TFIN], BF16, isOutput=False)

    logits = nc.declare_dram_parameter("logits", [NTT, P, VH], F32, isOutput=True)
    embT = nc.declare_dram_parameter("embT", [P, ND, S], F32, isOutput=True)
    mstat = nc.declare_dram_parameter("mstat", [P, NTT], F32, isOutput=True)
    sstat = nc.declare_dram_parameter("sstat", [P, NTT], F32, isOutput=True)

    # ---- DRAM scratch ----
    kcache = nc.dram_tensor("kcache", [L, H // 2, P, S], BF16)   # pair-feature x pos
    vcache = nc.dram_tensor("vcache", [L, H, S, DH], BF16)       # pos x dh
    hidT = nc.dram_tensor("hidT", [P, ND, S], F32)

    from contextlib import ExitStack
    with tile.TileContext(nc) as tc, ExitStack() as stack:
        cst = stack.enter_context(tc.tile_pool(name="cst", bufs=1))
        pw = stack.enter_context(tc.tile_pool(name="pw", bufs=1))
        pwd = stack.enter_context(tc.tile_pool(name="pwd", bufs=2))  # streamed slabs
        pa = stack.enter_context(tc.tile_pool(name="pa", bufs=1))    # big activations
        pt = stack.enter_context(tc.tile_pool(name="pt", bufs=2))    # small transients
        pt1 = stack.enter_context(tc.tile_pool(name="pt1", bufs=1))  # serial scalars
        ps = stack.enter_context(tc.tile_pool(name="ps", bufs=1, space="PSUM"))

        # ---- constants ----
        ones_col = cst.tile([P, 1], F32)
        nc.gpsimd.memset(ones_col[:], 1.0)
        ones_row = cst.tile([1, P], F32)
        nc.gpsimd.memset(ones_row[:], 1.0)
        ones_col_bf = cst.tile([P, 1], BF16)
        nc.gpsimd.memset(ones_col_bf[:], 1.0)
        identbf = cst.tile([P, P], BF16)
        make_identity(nc, identbf[:])
        epsb = cst.tile([1, 1], F32)
        nc.gpsimd.memset(epsb[:], LN_EPS)
        lnpt = cst.tile([P, (4 * L + 2) * ND], F32)
        for i in range(4 * L + 2):
            nc.sync.dma_start(lnpt[:, i * ND:(i + 1) * ND], lnp[i])

        def lnslice(i):  # [P, ND] view of param i
            return lnpt[:, i * ND:(i + 1) * ND]

        # persistent embeddings (scatter target)
        X0 = cst.tile([P, ND, S], F32)
        for j in range(ND):
            nc.sync.dma_start(X0[:, j, :], x0T[:, j, :])

        # masks
        mpre_sb = cst.tile([TPRE, TPRE], BF16)
        nc.sync.dma_start(mpre_sb[:], mpre[:])
        mfin_sb = cst.tile([P, 4, TFIN], BF16)
        for kt in range(4):
            nc.sync.dma_start(mfin_sb[:, kt, :], mfin[kt])

        # ---------------- layer norm ----------------
        def layer_norm(x_ap, T, gi, bi, out_ap, tagp=""):
            s1 = ps.tile([1, 512], F32, tag="pb0")
            s2 = ps.tile([1, 512], F32, tag="pb1")
            for j in range(ND):
                nc.tensor.matmul(s1[0:1, 0:T], ones_col[:], x_ap[:, j, :],
                                 start=(j == 0), stop=(j == ND - 1))
            for j in range(ND):
                xsq = pt.tile([P, TFIN], F32, tag="ln_xsq")
                nc.scalar.square(xsq[:, 0:T], x_ap[:, j, :])
                nc.tensor.matmul(s2[0:1, 0:T], ones_col[:], xsq[:, 0:T],
                                 start=(j == 0), stop=(j == ND - 1))
            mt = pt1.tile([1, TFIN], F32, tag="ln_m", name="ln_m")
            ta = pt1.tile([1, TFIN], F32, tag="ln_va", name="ln_va")
            tb = pt1.tile([1, TFIN], F32, tag="ln_vb", name="ln_vb")
            rt = pt1.tile([1, TFIN], F32, tag="ln_r", name="ln_r")
            m, r = mt[0:1, 0:T], rt[0:1, 0:T]
            nc.scalar.mul(m, s1[0:1, 0:T], 1.0 / D)
            nc.scalar.activation(ta[0:1, 0:T], s2[0:1, 0:T], AF.Identity, scale=1.0 / D)
            nc.scalar.square(tb[0:1, 0:T], m)
            nc.vector.tensor_sub(ta[0:1, 0:T], ta[0:1, 0:T], tb[0:1, 0:T])
            nc.scalar.activation(tb[0:1, 0:T], ta[0:1, 0:T], AF.Sqrt, bias=epsb[:])
            nc.vector.reciprocal(r, tb[0:1, 0:T])
            mrep = ps.tile([P, 512], F32, tag="pb2")
            rrep = ps.tile([P, 512], F32, tag="pb3")
            nc.tensor.matmul(mrep[:, 0:T], ones_row[:], m, start=True, stop=True)
            nc.tensor.matmul(rrep[:, 0:T], ones_row[:], r, start=True, stop=True)
            g_ap, b_ap = lnslice(gi), lnslice(bi)
            for j in range(ND):
                t1 = pt.tile([P, TFIN], F32, tag="ln_t1")
                nc.vector.tensor_sub(t1[:, 0:T], x_ap[:, j, :], mrep[:, 0:T])
                nc.vector.tensor_mul(t1[:, 0:T], t1[:, 0:T], rrep[:, 0:T])
                nc.scalar.activation(out_ap[:, j, :], t1[:, 0:T], AF.Identity,
                                     scale=g_ap[:, j:j + 1], bias=b_ap[:, j:j + 1])

        # ---------------- one pass ----------------
        def run_pass(pi, t0, T):
            kv_end = t0 + T
            nkt = (kv_end + P - 1) // P
            is_final = (pi == len(PASSES) - 1)

            X = pa.tile([P, ND, TFIN], F32, tag="X")
            for j in range(ND):
                nc.sync.dma_start(X[:, j, 0:T], posT[:, j, t0:t0 + T])
                nc.vector.tensor_add(X[:, j, 0:T], X[:, j, 0:T], X0[:, j, t0:t0 + T])

            for l in range(L):
                # ---- LN1 ----
                XLN = pa.tile([P, ND, TFIN], BF16, tag="XLN")
                layer_norm(X[:, :, 0:T], T, 4 * l, 4 * l + 1, XLN[:, :, 0:T])

                # ---- QKV (two column-halves of the weight resident) ----
                Q = pa.tile([P, ND, TFIN], BF16, tag="Q")
                for half in range(2):
                    wbig = pw.tile([P, ND, 2048], BF16, tag="wbig")
                    for k in range(ND):
                        nc.sync.dma_start(wbig[:, k, 0:1536],
                                          aqkv[l * ND + k][:, half * 1536:(half + 1) * 1536])
                    for mi in range(12):
                        m = half * 12 + mi
                        acc = ps.tile([P, 512], F32, tag=f"pb{mi % 4}")
                        for k in range(ND):
                            nc.tensor.matmul(acc[:, 0:T],
                                             wbig[:, k, mi * 128:(mi + 1) * 128],
                                             XLN[:, k, 0:T],
                                             start=(k == 0), stop=(k == ND - 1))
                        if m < 8:  # Q
                            nc.scalar.copy(Q[:, m, 0:T], acc[:, 0:T])
                        elif m < 16:  # K -> kcache[l, m-8, :, t0:kv_end]
                            kstg = pt.tile([P, TFIN], BF16, tag="kstg")
                            nc.scalar.copy(kstg[:, 0:T], acc[:, 0:T])
                            nc.sync.dma_start(kcache[l, m - 8, :, t0:t0 + T], kstg[:, 0:T])
                        else:  # V -> transpose -> vcache[l, h, pos, dh]
                            j = m - 16
                            vtmp = pt.tile([P, TFIN], BF16, tag="vtmp")
                            nc.scalar.copy(vtmp[:, 0:T], acc[:, 0:T])
                            for tb in range((T + P - 1) // P):
                                tl = min(P, T - tb * P)
                                tp = ps.tile([P, P], BF16, tag="pb6")
                                nc.tensor.transpose(tp[0:tl, :], vtmp[:, tb * P:tb * P + tl],
                                                    identbf[:])
                                vst = pt.tile([P, P], BF16, tag="vst")
                                nc.scalar.copy(vst[0:tl, :], tp[0:tl, :])
                                s0 = t0 + tb * P
                                nc.sync.dma_start(vcache[l, 2 * j, s0:s0 + tl, :],
                                                  vst[0:tl, 0:DH])
                                nc.sync.dma_start(vcache[l, 2 * j + 1, s0:s0 + tl, :],
                                                  vst[0:tl, DH:P])

                # ---- attention ----
                O = pa.tile([P, ND, TFIN], BF16, tag="O")
                for hp in range(ND):
                    dnA = ps.tile([1, 512], F32, tag="pb2")
                    dnB = ps.tile([1, 512], F32, tag="pb3")
                    ov = ps.tile([P, 512], F32, tag="pb4")
                    for kt in range(nkt):
                        klen = min(P, kv_end - kt * P)
                        kst = pt.tile([P, P], BF16, tag="kst")
                        nc.sync.dma_start(kst[:, 0:klen], kcache[l, hp, :, kt * P:kt * P + klen])
                        sA = ps.tile([P, 512], F32, tag="pb0")
                        sB = ps.tile([P, 512], F32, tag="pb1")
                        nc.tensor.matmul(sA[0:klen, 0:T], kst[0:DH, 0:klen],
                                         Q[0:DH, hp, 0:T], start=True, stop=True)
                        nc.tensor.matmul(sB[0:klen, 0:T], kst[DH:P, 0:klen],
                                         Q[DH:P, hp, 0:T], start=True, stop=True)
                        eA = pt.tile([P, TFIN], BF16, tag="eA")
                        eB = pt.tile([P, TFIN], BF16, tag="eB")
                        sc = 1.0 / math.sqrt(DH)
                        nc.scalar.activation(eA[0:klen, 0:T], sA[0:klen, 0:T], AF.Exp, scale=sc)
                        nc.scalar.activation(eB[0:klen, 0:T], sB[0:klen, 0:T], AF.Exp, scale=sc)
                        if is_final:
                            nc.vector.tensor_mul(eA[0:klen, 0:T], eA[0:klen, 0:T],
                                                 mfin_sb[0:klen, kt, 0:T])
                            nc.vector.tensor_mul(eB[0:klen, 0:T], eB[0:klen, 0:T],
                                                 mfin_sb[0:klen, kt, 0:T])
                        elif pi == 0:
                            nc.vector.tensor_mul(eA[0:klen, 0:T], eA[0:klen, 0:T],
                                                 mpre_sb[0:klen, 0:T])
                            nc.vector.tensor_mul(eB[0:klen, 0:T], eB[0:klen, 0:T],
                                                 mpre_sb[0:klen, 0:T])
                        nc.tensor.matmul(dnA[0:1, 0:T], ones_col_bf[0:klen, :], eA[0:klen, 0:T],
                                         start=(kt == 0), stop=(kt == nkt - 1))
                        nc.tensor.matmul(dnB[0:1, 0:T], ones_col_bf[0:klen, :], eB[0:klen, 0:T],
                                         start=(kt == 0), stop=(kt == nkt - 1))
                        vstA = pt.tile([P, DH], BF16, tag="vstA")
                        vstB = pt.tile([P, DH], BF16, tag="vstB")
                        nc.sync.dma_start(vstA[0:klen, :], vcache[l, 2 * hp, kt * P:kt * P + klen, :])
                        nc.sync.dma_start(vstB[0:klen, :], vcache[l, 2 * hp + 1, kt * P:kt * P + klen, :])
                        nc.tensor.matmul(ov[0:DH, 0:T], vstA[0:klen, :], eA[0:klen, 0:T],
                                         start=(kt == 0), stop=(kt == nkt - 1),
                                         skip_group_check=True)
                        nc.tensor.matmul(ov[DH:P, 0:T], vstB[0:klen, :], eB[0:klen, 0:T],
                                         start=(kt == 0), stop=(kt == nkt - 1),
                                         skip_group_check=True)
                    rA = pt1.tile([1, TFIN], F32, tag="rA", name="rA")
                    rB = pt1.tile([1, TFIN], F32, tag="rB", name="rB")
                    nc.vector.reciprocal(rA[0:1, 0:T], dnA[0:1, 0:T])
                    nc.vector.reciprocal(rB[0:1, 0:T], dnB[0:1, 0:T])
                    rrep = ps.tile([P, 512], F32, tag="pb5")
                    nc.tensor.matmul(rrep[0:DH, 0:T], ones_row[0:1, 0:DH], rA[0:1, 0:T],
                                     start=True, stop=True)
                    nc.tensor.matmul(rrep[DH:P, 0:T], ones_row[0:1, 0:DH], rB[0:1, 0:T],
                                     start=True, stop=True)
                    rrs = pt.tile([P, TFIN], F32, tag="rrs")
                    nc.scalar.copy(rrs[:, 0:T], rrep[:, 0:T])
                    nc.vector.tensor_mul(O[:, hp, 0:T], ov[:, 0:T], rrs[:, 0:T])

                # ---- Wo (k-outer, 8 live psums) + residual ----
                wops = [ps.tile([P, 512], F32, tag=f"pb{m}", name=f"wops{m}") for m in range(8)]
                for k in range(ND):
                    wos = pwd.tile([P, D], BF16, tag="wo")
                    nc.sync.dma_start(wos[:], ao[l * ND + k])
                    for m in range(ND):
                        nc.tensor.matmul(wops[m][:, 0:T], wos[:, m * 128:(m + 1) * 128],
                                         O[:, k, 0:T], start=(k == 0), stop=(k == ND - 1))
                for m in range(ND):
                    nc.vector.tensor_add(X[:, m, 0:T], X[:, m, 0:T], wops[m][:, 0:T])

                # ---- LN2 ----
                XL2 = pa.tile([P, ND, TFIN], BF16, tag="XLN")
                layer_norm(X[:, :, 0:T], T, 4 * l + 2, 4 * l + 3, XL2[:, :, 0:T])

                # ---- W1 + gelu (two column-halves resident) ----
                H1 = pa.tile([P, NH1, TFIN], BF16, tag="H1")
                for half in range(2):
                    wbig = pw.tile([P, ND, 2048], BF16, tag="wbig")
                    for k in range(ND):
                        nc.sync.dma_start(wbig[:, k, :],
                                          a1[l * ND + k][:, half * 2048:(half + 1) * 2048])
                    for mi in range(16):
                        m = half * 16 + mi
                        acc = ps.tile([P, 512], F32, tag=f"pb{mi % 4}")
                        for k in range(ND):
                            nc.tensor.matmul(acc[:, 0:T], wbig[:, k, mi * 128:(mi + 1) * 128],
                                             XL2[:, k, 0:T], start=(k == 0), stop=(k == ND - 1))
                        gs = pt.tile([P, TFIN], F32, tag="gelu_s")
                        gu = pt.tile([P, TFIN], F32, tag="gelu_u")
                        nc.scalar.activation(gs[:, 0:T], acc[:, 0:T], AF.Square, scale=GC1)
                        nc.vector.tensor_mul(gu[:, 0:T], acc[:, 0:T], gs[:, 0:T])
                        nc.vector.tensor_add(gu[:, 0:T], gu[:, 0:T], acc[:, 0:T])
                        nc.scalar.activation(gs[:, 0:T], gu[:, 0:T], AF.Sigmoid, scale=GC2)
                        nc.vector.tensor_mul(H1[:, m, 0:T], acc[:, 0:T], gs[:, 0:T])

                # ---- W2 (k-outer, 8 live psums) + residual ----
                w2ps = [ps.tile([P, 512], F32, tag=f"pb{m}", name=f"w2ps{m}") for m in range(8)]
                for k in range(NH1):
                    w2s = pwd.tile([P, D], BF16, tag="w2")
                    nc.sync.dma_start(w2s[:], a2[l * NH1 + k])
                    for m in range(ND):
                        nc.tensor.matmul(w2ps[m][:, 0:T], w2s[:, m * 128:(m + 1) * 128],
                                         H1[:, k, 0:T], start=(k == 0), stop=(k == NH1 - 1))
                for m in range(ND):
                    nc.vector.tensor_add(X[:, m, 0:T], X[:, m, 0:T], w2ps[m][:, 0:T])

            # ---- final LN -> hid ----
            hid = pa.tile([P, ND, TFIN], F32, tag="hid")
            layer_norm(X[:, :, 0:T], T, 4 * L, 4 * L + 1, hid[:, :, 0:T])
            for j in range(ND):
                nc.sync.dma_start(hidT[:, j, t0:t0 + T], hid[:, j, 0:T])
            if not is_final:
                for j in range(ND):
                    nc.vector.tensor_copy(X0[:, j, t0 + T:t0 + T + 1], hid[:, j, T - 1:T])

        for pi, (t0, T) in enumerate(PASSES):
            run_pass(pi, t0, T)

        # ---------------- lm_head + streaming softmax stats ----------------
        hidbf = cst.tile([P, ND, S], BF16)
        hf = pa.tile([P, ND, TFIN], F32, tag="X")
        for j in range(ND):
            nc.sync.dma_start(hf[:, j, 0:S // 2], hidT[:, j, 0:S // 2])
            nc.vector.tensor_copy(hidbf[:, j, 0:S // 2], hf[:, j, 0:S // 2])
        hf2 = pa.tile([P, ND, TFIN], F32, tag="hid")
        for j in range(ND):
            nc.sync.dma_start(hf2[:, j, 0:S // 2], hidT[:, j, S // 2:S])
            nc.vector.tensor_copy(hidbf[:, j, S // 2:S], hf2[:, j, 0:S // 2])

        mst = cst.tile([P, NTT], F32)
        sst = cst.tile([P, NTT], F32)
        nc.gpsimd.memset(mst[:], -3.0e38)
        nc.gpsimd.memset(sst[:], 0.0)

        for v in range(NVC):
            lms = pwd.tile([P, ND, VCHUNK], BF16, tag="lm")
            for k in range(ND):
                nc.sync.dma_start(lms[:, k, :], lmw[k][:, v * VCHUNK:(v + 1) * VCHUNK])
            for tt in range(NTT):
                acc = ps.tile([P, 512], F32, tag=f"pb{tt % 4}")
                for k in range(ND):
                    nc.tensor.matmul(acc[:, 0:VCHUNK], hidbf[:, k, tt * P:(tt + 1) * P],
                                     lms[:, k, :], start=(k == 0), stop=(k == ND - 1))
                lsb = pt.tile([P, VCHUNK], F32, tag="lsb")
                nc.scalar.copy(lsb[:], acc[:, 0:VCHUNK])
                nc.sync.dma_start(logits[tt][:, v * VCHUNK:(v + 1) * VCHUNK], lsb[:])
                tmax = pt.tile([P, 1], F32, tag="tmax")
                nc.vector.reduce_max(tmax[:], acc[:, 0:VCHUNK], axis=mybir.AxisListType.X)
                mnew = pt.tile([P, 1], F32, tag="mnew")
                nc.vector.tensor_max(mnew[:], mst[:, tt:tt + 1], tmax[:])
                dm = pt.tile([P, 1], F32, tag="dm")
                nc.vector.tensor_sub(dm[:], mst[:, tt:tt + 1], mnew[:])
                scl = pt.tile([P, 1], F32, tag="scl")
                nc.scalar.activation(scl[:], dm[:], AF.Exp)
                nc.vector.tensor_mul(sst[:, tt:tt + 1], sst[:, tt:tt + 1], scl[:])
                nm = pt.tile([P, 1], F32, tag="nm")
                nc.scalar.activation(nm[:], mnew[:], AF.Identity, scale=-1.0)
                esc = pt.tile([P, VCHUNK], F32, tag="esc")
                esum = pt.tile([P, 1], F32, tag="esum")
                nc.scalar.activation(esc[:], acc[:, 0:VCHUNK], AF.Exp, bias=nm[:],
                                     accum_out=esum[:])
                nc.vector.tensor_add(sst[:, tt:tt + 1], sst[:, tt:tt + 1], esum[:])
                nc.vector.tensor_copy(mst[:, tt:tt + 1], mnew[:])

        nc.sync.dma_start(mstat[:], mst[:])
        nc.sync.dma_start(sstat[:], sst[:])
        for j in range(ND):
            nc.sync.dma_start(embT[:, j, :], X0[:, j, :])

    nc.compile()
    return nc


# ---------------------------------------------------------------------------
# host side
# ---------------------------------------------------------------------------
_NC_CACHE = {}


def _feature_major(a):
    """[T, D] f32 -> [128, 8, T]"""
    T = a.shape[0]
    return np.ascontiguousarray(a.reshape(T, ND, P).transpose(2, 1, 0))


def _vec_fm(v):
    """[D] -> [128, 8]"""
    return np.ascontiguousarray(v.reshape(ND, P).T)


def _prep_inputs(input_ids, position_ids, emb, pos_emb, Wqkv, Wo, W1, W2,
                 ln1_g, ln1_b, ln2_g, ln2_b, lnf_g, lnf_b, lm_head):
    bf = ml_dtypes.bfloat16
    aqkv = np.ascontiguousarray(
        Wqkv.reshape(L, ND, P, 3 * D)).reshape(L * ND, P, 3 * D).astype(bf)
    ao = np.ascontiguousarray(
        Wo.reshape(L, ND, P, D)).reshape(L * ND, P, D).astype(bf)
    a1 = np.ascontiguousarray(
        W1.reshape(L, ND, P, 4 * D)).reshape(L * ND, P, 4 * D).astype(bf)
    a2 = np.ascontiguousarray(
        W2.reshape(L, NH1, P, D)).reshape(L * NH1, P, D).astype(bf)

    lnp = np.zeros((4 * L + 2, P, ND), np.float32)
    for l in range(L):
        lnp[4 * l + 0] = _vec_fm(ln1_g[l]); lnp[4 * l + 1] = _vec_fm(ln1_b[l])
        lnp[4 * l + 2] = _vec_fm(ln2_g[l]); lnp[4 * l + 3] = _vec_fm(ln2_b[l])
    lnp[4 * L] = _vec_fm(lnf_g); lnp[4 * L + 1] = _vec_fm(lnf_b)

    # masks (multiplicative, on exp(S^T) [k, q])
    q = np.arange(TPRE)
    mpre_np = (np.arange(TPRE)[:, None] <= q[None, :]).astype(bf)
    ind2_np = np.zeros((2, P), np.float32)
    ind2_np[0, 0:DH] = 1.0
    ind2_np[1, DH:P] = 1.0
    qf = T0FIN + np.arange(TFIN)
    kf = np.arange(4 * P).reshape(4, P)
    mfin_np = (kf[:, :, None] <= qf[None, None, :]).astype(bf)

    per_core = []
    for c in range(8):
        b = c % 4
        vh = c // 4
        x_emb = emb[input_ids[b]]                       # [S, D]
        posE = pos_emb[position_ids[b]]                 # [S, D]
        lmw = np.ascontiguousarray(
            lm_head[:, vh * VH:(vh + 1) * VH].reshape(ND, P, VH)).astype(bf)
        per_core.append({
            "x0T": _feature_major(x_emb), "posT": _feature_major(posE),
            "lnp": lnp, "aqkv": aqkv, "ao": ao, "a1": a1, "a2": a2,
            "lmw": lmw, "mpre": mpre_np, "mfin": mfin_np, "ind2h": ind2_np,
        })
    return per_core


def kernel(input_ids, attention_mask, labels, position_ids, emb, pos_emb,
           Wqkv, Wo, W1, W2, ln1_g, ln1_b, ln2_g, ln2_b, lnf_g, lnf_b, lm_head):
    input_ids = np.asarray(input_ids); position_ids = np.asarray(position_ids)
    labels = np.asarray(labels)
    emb = np.asarray(emb, np.float32); pos_emb = np.asarray(pos_emb, np.float32)
    args = [np.asarray(a, np.float32) for a in
            (Wqkv, Wo, W1, W2, ln1_g, ln1_b, ln2_g, ln2_b, lnf_g, lnf_b, lm_head)]

    if "nc" not in _NC_CACHE:
        _NC_CACHE["nc"] = build_device()
    nc = _NC_CACHE["nc"]

    in_maps = _prep_inputs(input_ids, position_ids, emb, pos_emb, *args)
    res = run_bass_kernel_spmd(nc, in_maps, list(range(8)))
    r = res.results

    # assemble logits [B, S, V]
    logits = np.empty((B, S, V), np.float32)
    embeds = np.empty((B, S, D), np.float32)
    mhalf = np.empty((2, B, S), np.float32)
    shalf = np.empty((2, B, S), np.float32)
    for c in range(8):
        b, vh = c % 4, c // 4
        lg = r[c]["logits"]                       # [4, 128, VH]
        logits[b, :, vh * VH:(vh + 1) * VH] = lg.reshape(S, VH)
        mhalf[vh, b] = r[c]["mstat"].T.reshape(S)
        shalf[vh, b] = r[c]["sstat"].T.reshape(S)
        if vh == 0:
            e = r[c]["embT"]                      # [128, 8, S]
            embeds[b] = e.transpose(2, 1, 0).reshape(S, D)

    # loss from stats + label gather
    m = np.maximum(mhalf[0], mhalf[1])            # [B, S]
    sume = shalf[0] * np.exp(mhalf[0] - m) + shalf[1] * np.exp(mhalf[1] - m)
    lse = m + np.log(sume)                        # [B, S]
    sl = labels[:, 1:]
    lab_logit = np.take_along_axis(logits[:, :-1], sl[..., None], axis=2)[..., 0]
    loss = np.float32(np.mean(lse[:, :-1] - lab_logit))

    return loss, embeds, logits
